# revision 16
# baseline (speedup 1.0000x reference)
"""Trainium2 Bass kernel for nn_Conv1dBlock (LIF spikes -> Conv1d(k=5, same) -> GroupNorm).

Contract: kernel(**inputs) takes FULL inputs (x [4,64,256,512] f32, conv_w
[256,256,5], conv_b/gamma/beta [256]) and returns the FULL [4,64,256,512] f32
output. Internally shards data-parallel over B across 8 NeuronCores.

Per-core algorithm (B_loc = 8):
  - LIF (DVE, fp32, u = 2*v scaling):
      u = 0.5*m + x ; s = (u >= 1) -> fp8 ; m = (u < 1)*u
  - Conv1d as fp8 DoubleRow matmuls (2x PE throughput vs bf16): weights
    e4m3 at scale 2^13; 5 single-precision tap matmuls (each contracting
    all 256 ci via DoubleRow) + 4 residual-correction matmuls for taps
    0-3. Tap 4's residual dropped: measured rel err 1.58e-2 (gate 2e-2).
  - GroupNorm stats split across engines per co-tile:
      ct0: DVE bn_stats/bn_aggr -> (mean, var)
      ct1: ScalarE Copy+accum (r=sum y) and Square+accum (q=sum y^2);
           the Copy doubles as the PSUM->SBUF move; Square writes PSUM
           in place (last use).
    Group combine via ONE block-diagonal [128,128] f32 matmul whose
    output is the per-group sums already broadcast to every channel.
  - Affine out = A*Y + B on ScalarE (Identity with per-channel scale/bias
    APs) from the SBUF f32 copy, writing fp16; DMA out fp16 (host upcasts).
  - Software pipelining: LIF(i) is emitted before stats(i-1) so spikes
    (which gate the PE) never queue behind stats work on DVE.
"""

import numpy as np
import ml_dtypes

T, B_FULL, C, L, K = 4, 64, 256, 512, 5
N_CORES = 8
B_LOC = B_FULL // N_CORES
G = 8            # groups
GPC = C // G     # 32 channels per group
CT = 2           # 128-channel tiles
EPS = 1e-5
WSCALE = 2.0 ** 13
EPS_S = EPS * WSCALE * WSCALE
NR = 4           # residual-corrected taps (0..3)

_COMPILED = {}


def _build_program():
    import concourse.bass as bass
    import concourse.tile as tile
    from concourse import bacc, mybir

    f32 = mybir.dt.float32
    bf16 = mybir.dt.bfloat16
    fp8 = mybir.dt.float8e4
    f16 = mybir.dt.float16
    Alu = mybir.AluOpType
    Act = mybir.ActivationFunctionType
    DR = mybir.MatmulPerfMode.DoubleRow

    nc = bacc.Bacc(
        "TRN2",
        target_bir_lowering=False,
        debug=False,
        num_devices=N_CORES,
    )

    x_d = nc.dram_tensor("x", [T, B_LOC, C, L], f32, kind="ExternalInput").ap()
    # [ci, k, co_t, ci_t, co] single e4m3 at scale 2^13
    ws_d = nc.dram_tensor("ws", [128, K, CT, 2, 128], fp8, kind="ExternalInput").ap()
    # residuals for taps 0..NR-1, same layout/scale
    wr_d = nc.dram_tensor("wr", [128, NR, CT, 2, 128], fp8, kind="ExternalInput").ap()
    # [co, field, m(rep), co_t]; fields: b', gamma, beta, 2b', b'^2  (b'=2^13 b)
    chan_d = nc.dram_tensor("chan", [128, 5, 2, CT], f32, kind="ExternalInput").ap()
    # block-diagonal group-sum-broadcast matrix (32-channel blocks of ones)
    onesgg_d = nc.dram_tensor("onesgg", [128, 128], f32, kind="ExternalInput").ap()
    y_d = nc.dram_tensor("y", [T, B_LOC, C, L], f16, kind="ExternalOutput").ap()

    with tile.TileContext(nc) as tc:
        with (
            tc.tile_pool(name="singles", bufs=1) as singles,
            tc.tile_pool(name="xp", bufs=10) as xp,
            tc.tile_pool(name="sp", bufs=6) as sp,
            tc.tile_pool(name="ysb", bufs=8) as ysb,
            tc.tile_pool(name="ysb32", bufs=8) as ysb32,
            tc.tile_pool(name="smallsb", bufs=12) as smallsb,
            tc.tile_pool(name="ypsum", bufs=6, space="PSUM") as ypsum,
            tc.tile_pool(name="spsum", bufs=2, space="PSUM") as spsum,
        ):
            # PE p-state warmup (overlaps startup DMA; keeps clock ramping)
            warm_sb = singles.tile([128, 64], bf16)
            nc.vector.memset(warm_sb[:], 0.25)
            warm_ps = spsum.tile([128, 32], f32, name="small_ps")
            for _ in range(60):
                nc.tensor.matmul(
                    warm_ps[0:64, 0:8], warm_sb[:, 0:64], warm_sb[:, 0:8],
                    start=True, stop=True, skip_group_check=True,
                )
            first_small_ps = warm_ps
            # first x tiles split in halves across queues (gate LIF(0)),
            # then weights (smaller, land in parallel)
            early_x = {}
            for b in range(2):
                xt = xp.tile([128, 2, L], f32)
                xsrc = x_d[0, b].rearrange("(i p) l -> p i l", p=128)
                for h in range(2):
                    for q in range(4):
                        sl = slice(q * (L // 4), (q + 1) * (L // 4))
                        nc.sync.dma_start(
                            out=xt[:, h : h + 1, sl], in_=xsrc[:, h : h + 1, sl]
                        )
                early_x[(0, b)] = xt
            ws = singles.tile([128, K, CT, 2, 128], fp8)
            for k in range(K):
                nc.sync.dma_start(out=ws[:, k], in_=ws_d[:, k])
            wr = singles.tile([128, NR, CT, 2, 128], fp8)
            for k in range(NR):
                nc.sync.dma_start(out=wr[:, k], in_=wr_d[:, k])
            onesgg = singles.tile([128, 128], f32)
            nc.sync.dma_start(out=onesgg[:], in_=onesgg_d[:])
            chan = singles.tile([128, 5, 2, CT], f32)
            nc.sync.dma_start(out=chan[:], in_=chan_d[:])
            eps_t = singles.tile([128, 1], f32)
            nc.vector.memset(eps_t[:], EPS_S)
            # pre-load the activation table (Sqrt selects sqrt_and_others,
            # which also holds Copy/Identity/Square) off the critical path
            eps_s = singles.tile([128, 1], f32)
            nc.scalar.activation(out=eps_s[0:1], in_=eps_t[0:1], func=Act.Sqrt)

            # persistent LIF membrane state (u = 2v scaling) per local batch
            m_tiles = []
            for b in range(B_LOC):
                mt = singles.tile([128, 2, L], f32, tag=f"m{b}")
                m_tiles.append(mt)

            # tap -> (rhs_lo, rhs_hi, out_lo, out_hi) column ranges
            tap_slices = []
            for k in range(K):
                d = k - 2
                if d >= 0:
                    tap_slices.append((d, L, 0, L - d))
                else:
                    tap_slices.append((0, L + d, -d, L))

            mm_list = [("s", 2), ("s", 0), ("s", 1), ("s", 3), ("s", 4)]
            mm_list += [("r", j) for j in range(NR)]
            n_mm = len(mm_list)

            INV_L = 1.0 / L
            INV_G = 1.0 / GPC

            def flush_stats(fs):
                """Stats + PSUM drain for one sample, one iteration later.
                ct0: DVE bn_stats/aggr -> (mean, var); ACT copy -> SBUF.
                ct1: ACT copy+accum (r) -> SBUF; ACT square+accum (q) in PSUM.
                """
                cur, mi, yps, y32s = fs
                stats = cur[2]
                # ct0
                bns = smallsb.tile([128, 6], f32)
                nc.vector.bn_stats(out=bns[:], in_=yps[0][:])
                nc.vector.bn_aggr(out=stats[:, mi, 0, :], in_=bns[:])
                nc.scalar.activation(out=y32s[0][:], in_=yps[0][:], func=Act.Copy)
                # ct1
                nc.scalar.activation(
                    out=y32s[1][:], in_=yps[1][:], func=Act.Copy,
                    accum_out=stats[:, mi, 1, 0:1],
                )
                nc.scalar.activation(
                    out=yps[1][:], in_=yps[1][:], func=Act.Square,
                    accum_out=stats[:, mi, 1, 1:2],
                )

            def tail_front(pend):
                """a/z per-channel stats (DVE) + group-sum-broadcast matmul."""
                tb_pair, small_ps, stats, az, y32all = pend
                gm = stats.shape[1]
                # ct0: a = mean + b' ; z = a^2 + var
                a0 = az[:, :, 0, 0]
                nc.vector.tensor_add(out=a0, in0=stats[:, :, 0, 0], in1=chan[:, 0, 0:gm, 0])
                m0 = smallsb.tile([128, gm], f32)
                nc.vector.tensor_mul(out=m0[:], in0=a0, in1=a0)
                nc.vector.tensor_add(out=az[:, :, 0, 1], in0=m0[:], in1=stats[:, :, 0, 1])
                # ct1: mean = r/L ; a = mean + b' ; z = q/L + b'*(mean + a)
                mc = smallsb.tile([128, gm, 2], f32)
                nc.vector.tensor_scalar(
                    out=mc[:, :, 0], in0=stats[:, :, 1, 0], scalar1=INV_L,
                    scalar2=None, op0=Alu.mult,
                )
                a1 = az[:, :, 1, 0]
                nc.vector.tensor_add(out=a1, in0=mc[:, :, 0], in1=chan[:, 0, 0:gm, 1])
                u1 = smallsb.tile([128, gm, 2], f32)
                nc.vector.tensor_add(out=u1[:, :, 0], in0=mc[:, :, 0], in1=a1)
                nc.vector.tensor_mul(out=u1[:, :, 1], in0=u1[:, :, 0], in1=chan[:, 0, 0:gm, 1])
                nc.vector.scalar_tensor_tensor(
                    out=az[:, :, 1, 1], in0=stats[:, :, 1, 1], scalar=INV_L,
                    in1=u1[:, :, 1], op0=Alu.mult, op1=Alu.add,
                )
                # group sums broadcast to channels: [128, gm*CT*2]
                nc.tensor.matmul(
                    small_ps[:, 0 : gm * 4],
                    onesgg[:],
                    az[:].rearrange("p m c s -> p (m c s)"),
                    start=True, stop=True,
                )

            def tail_mid(pend):
                """mu/kappa chain + A/B coefficients (all 128-partition)."""
                tb_pair, small_ps, stats, az, y32all = pend
                gm = stats.shape[1]
                gsb = small_ps[:, 0 : gm * 4].rearrange("p (m c s) -> p m c s", m=gm, c=CT)
                mu = smallsb.tile([128, gm, CT], f32)
                m2 = smallsb.tile([128, gm, CT], f32)
                vr = smallsb.tile([128, gm, CT], f32)
                kp = smallsb.tile([128, gm, CT], f32)
                nc.vector.tensor_scalar(
                    out=mu[:], in0=gsb[:, :, :, 0], scalar1=INV_G,
                    scalar2=None, op0=Alu.mult,
                )
                nc.vector.tensor_mul(out=m2[:], in0=mu[:], in1=mu[:])
                nc.vector.scalar_tensor_tensor(
                    out=vr[:], in0=gsb[:, :, :, 1], scalar=INV_G, in1=m2[:],
                    op0=Alu.mult, op1=Alu.subtract,
                )
                nc.scalar.activation(
                    out=vr[:], in_=vr[:], func=Act.Sqrt, bias=eps_t[:],
                )
                nc.vector.reciprocal(out=kp[:], in_=vr[:])
                # A = kappa * gamma ; B = (b' - mu) * A + beta  (GpSimd)
                ab = smallsb.tile([128, gm, CT, 2], f32)
                tmp = smallsb.tile([128, gm, CT], f32)
                nc.vector.tensor_mul(out=ab[:, :, :, 0], in0=kp[:], in1=chan[:, 1, 0:gm])
                nc.vector.tensor_sub(out=tmp[:], in0=chan[:, 0, 0:gm], in1=mu[:])
                nc.vector.tensor_mul(out=tmp[:], in0=tmp[:], in1=ab[:, :, :, 0])
                nc.vector.tensor_add(out=ab[:, :, :, 1], in0=tmp[:], in1=chan[:, 2, 0:gm])
                return ab

            def tail_store(pend, ab):
                """out = A*Y + B on ScalarE (f32 SBUF -> fp16 SBUF), DMA out."""
                tb_pair, small_ps, stats, az, y32all = pend
                for mi in range(len(tb_pair)):
                    t, b = tb_pair[mi]
                    last = t == T - 1 and b == B_LOC - 1
                    for ct in range(CT):
                        y_sb = ysb.tile([128, L], f16)
                        nc.scalar.activation(
                            out=y_sb[:], in_=y32all[mi][ct][:], func=Act.Identity,
                            scale=ab[:, mi, ct, 0:1], bias=ab[:, mi, ct, 1:2],
                        )
                        dst = y_d[t, b].rearrange("(i p) l -> p i l", p=128)[:, ct, :]
                        if last:
                            # halves on separate queues to shorten the drain
                            nc.sync.dma_start(out=dst[:, 0 : L // 2], in_=y_sb[:, 0 : L // 2])
                            nc.sync.dma_start(out=dst[:, L // 2 : L], in_=y_sb[:, L // 2 : L])
                        else:
                            nc.sync.dma_start(out=dst, in_=y_sb[:])

            groups = [(2 * i, 2 * i + 1) for i in range(12)] + [(s,) for s in range(24, 32)]
            gof = {}
            for g in groups:
                for j, s_ in enumerate(g):
                    gof[s_] = (g, j)
            pend_stats = None
            pend_reset = None
            pending = None     # group with stats complete, awaiting front
            pend_mid = None    # group with gsum done, awaiting mid+store
            cur = None
            for t in range(T):
                for b in range(B_LOC):
                    idx = t * B_LOC + b
                    grp, mi = gof[idx]
                    gsz = len(grp)
                    if mi == 0:
                        if idx == 0:
                            small_ps = first_small_ps
                        else:
                            small_ps = spsum.tile([128, 32], f32, name="small_ps")
                        stats = smallsb.tile([128, gsz, CT, 2], f32, name="stats")
                        az = smallsb.tile([128, gsz, CT, 2], f32, name="az")
                        cur = ([None] * gsz, small_ps, stats, az, [None] * gsz)
                    cur[0][mi] = (t, b)

                    # 1. LIF for this sample (feeds PE soonest)
                    xt = early_x.pop((t, b), None)
                    if xt is None:
                        xt = xp.tile([128, 2, L], f32)
                        nc.sync.dma_start(
                            out=xt[:],
                            in_=x_d[t, b].rearrange("(i p) l -> p i l", p=128),
                        )
                    mt = m_tiles[b]
                    st = sp.tile([128, 2, L], fp8)
                    if t == 0:
                        nc.vector.tensor_scalar(
                            out=st[:], in0=xt[:], scalar1=1.0, scalar2=None,
                            op0=Alu.is_ge,
                        )
                        nc.vector.scalar_tensor_tensor(
                            out=mt[:], in0=xt[:], scalar=1.0, in1=xt[:],
                            op0=Alu.is_lt, op1=Alu.mult,
                        )
                    else:
                        nc.vector.scalar_tensor_tensor(
                            out=mt[:], in0=mt[:], scalar=0.5, in1=xt[:],
                            op0=Alu.mult, op1=Alu.add,
                        )
                        nc.vector.tensor_scalar(
                            out=st[:], in0=mt[:], scalar1=1.0, scalar2=None,
                            op0=Alu.is_ge,
                        )

                    # 2. stats/drain for previous sample
                    if pend_stats is not None:
                        flush_stats(pend_stats)
                        fcur, fmi, _, fy32 = pend_stats
                        fcur[4][fmi] = fy32
                        if fmi == len(fcur[0]) - 1:
                            pending = fcur
                        pend_stats = None
                    # deferred membrane reset for the previous sample (keeps
                    # this sample's spike at the head of the DVE queue; m[b]
                    # is not needed again for 8 iterations)
                    if pend_reset is not None:
                        nc.vector.scalar_tensor_tensor(
                            out=pend_reset[:], in0=pend_reset[:], scalar=1.0,
                            in1=pend_reset[:], op0=Alu.is_lt, op1=Alu.mult,
                        )
                        pend_reset = None
                    if t < T - 1:
                        pend_reset = mt

                    # 3. finish the pair before that
                    if pend_mid is not None:
                        ab = tail_mid(pend_mid)
                        tail_store(pend_mid, ab)
                        pend_mid = None

                    # 4. conv for this sample
                    yps = []
                    y32s = []
                    for ct in range(CT):
                        yp = ypsum.tile([128, L], f32)
                        for i, (kind, k) in enumerate(mm_list):
                            rl, rh, ol, oh = tap_slices[k]
                            w_ap = ws[:, k, ct] if kind == "s" else wr[:, k, ct]
                            nc.tensor.matmul(
                                yp[:, ol:oh],
                                w_ap,
                                st[:, :, rl:rh],
                                start=(i == 0),
                                stop=(i == n_mm - 1),
                                perf_mode=DR,
                                skip_group_check=True,
                            )
                        yps.append(yp)
                        y32s.append(ysb32.tile([128, L], f32, name="y32"))
                    pend_stats = (cur, mi, yps, y32s)

                    # 5. group-sum matmul for the completed pair (after this
                    # sample's convs in the PE queue)
                    if mi == 0 and pending is not None:
                        tail_front(pending)
                        pend_mid = pending
                        pending = None

            # final drain
            flush_stats(pend_stats)
            fcur, fmi, _, fy32 = pend_stats
            fcur[4][fmi] = fy32
            if pend_mid is not None:
                ab = tail_mid(pend_mid)
                tail_store(pend_mid, ab)
            tail_front(fcur)
            ab = tail_mid(fcur)
            tail_store(fcur, ab)

    nc.compile()
    return nc


def _prep_host_inputs(x, conv_w, conv_b, gamma, beta):
    x = np.asarray(x, dtype=np.float32)
    conv_w = np.asarray(conv_w, dtype=np.float32)
    conv_b = np.asarray(conv_b, dtype=np.float32)
    gamma = np.asarray(gamma, dtype=np.float32)
    beta = np.asarray(beta, dtype=np.float32)

    def q8(a):
        return a.astype(ml_dtypes.float8_e4m3).astype(np.float32)

    # [ci_t, ci, co_t, co, k] at scale 2^13
    Wt = conv_w.transpose(1, 0, 2)                      # [ci_g, co_g, k]
    W6 = Wt.reshape(2, 128, CT, 128, K) * np.float32(WSCALE)
    w8 = q8(W6)
    r8 = q8(W6 - w8)
    # ws[ci, k, ct, ci_t, co]
    ws_host = np.ascontiguousarray(
        w8.transpose(1, 4, 2, 0, 3).astype(ml_dtypes.float8_e4m3)
    )
    # wr[ci, j(tap), ct, ci_t, co] for taps 0..NR-1
    wr_host = np.ascontiguousarray(
        r8[:, :, :, :, 0:NR].transpose(1, 4, 2, 0, 3).astype(ml_dtypes.float8_e4m3)
    )

    bp = conv_b * np.float32(WSCALE)
    fields = np.stack([bp, gamma, beta, 2.0 * bp, bp * bp])        # [5, 256]
    chan1 = fields.reshape(5, CT, 128).transpose(2, 0, 1)          # [128, 5, ct]
    chan = np.ascontiguousarray(
        np.broadcast_to(chan1[:, :, None, :], (128, 5, 2, CT))
    )

    onesgg = np.zeros((128, 128), np.float32)
    for ci in range(128):
        g0 = (ci // GPC) * GPC
        onesgg[ci, g0 : g0 + GPC] = 1.0

    shards = []
    for i in range(N_CORES):
        shards.append(
            {
                "x": np.ascontiguousarray(x[:, i * B_LOC : (i + 1) * B_LOC]),
                "ws": ws_host,
                "wr": wr_host,
                "chan": chan,
                "onesgg": onesgg,
            }
        )
    return shards


def kernel(x, conv_w, conv_b, gamma, beta, _trace=False):
    from concourse.bass_utils import run_bass_kernel_spmd

    if "nc" not in _COMPILED:
        _COMPILED["nc"] = _build_program()
    nc = _COMPILED["nc"]

    in_maps = _prep_host_inputs(x, conv_w, conv_b, gamma, beta)
    res = run_bass_kernel_spmd(
        nc, in_maps, list(range(N_CORES)), trace=_trace
    )
    out = np.concatenate([r["y"] for r in res.results], axis=1).astype(np.float32)
    _COMPILED["last_result"] = res
    return out


# revision 17
# speedup vs baseline: 1.0366x; 1.0366x over previous
"""Trainium2 Bass kernel for nn_Conv1dBlock (LIF spikes -> Conv1d(k=5, same) -> GroupNorm).

Contract: kernel(**inputs) takes FULL inputs (x [4,64,256,512] f32, conv_w
[256,256,5], conv_b/gamma/beta [256]) and returns the FULL [4,64,256,512] f32
output. Internally shards data-parallel over B across 8 NeuronCores.

Per-core algorithm (B_loc = 8):
  - LIF (DVE, fp32, u = 2*v scaling):
      u = 0.5*m + x ; s = (u >= 1) -> fp8 ; m = (u < 1)*u
  - Conv1d as fp8 DoubleRow matmuls (2x PE throughput vs bf16): weights
    e4m3 at scale 2^13; 5 single-precision tap matmuls (each contracting
    all 256 ci via DoubleRow) + 4 residual-correction matmuls for taps
    0-3. Tap 4's residual dropped: measured rel err 1.58e-2 (gate 2e-2).
  - GroupNorm stats split across engines per co-tile:
      ct0: DVE bn_stats/bn_aggr -> (mean, var)
      ct1: ScalarE Copy+accum (r=sum y) and Square+accum (q=sum y^2);
           the Copy doubles as the PSUM->SBUF move; Square writes PSUM
           in place (last use).
    Group combine via ONE block-diagonal [128,128] f32 matmul whose
    output is the per-group sums already broadcast to every channel.
  - Affine out = A*Y + B on ScalarE (Identity with per-channel scale/bias
    APs) from the SBUF f32 copy, writing fp16; DMA out fp16 (host upcasts).
  - Software pipelining: LIF(i) is emitted before stats(i-1) so spikes
    (which gate the PE) never queue behind stats work on DVE.
"""

import numpy as np
import ml_dtypes

T, B_FULL, C, L, K = 4, 64, 256, 512, 5
N_CORES = 8
B_LOC = B_FULL // N_CORES
G = 8            # groups
GPC = C // G     # 32 channels per group
CT = 2           # 128-channel tiles
EPS = 1e-5
WSCALE = 2.0 ** 13
EPS_S = EPS * WSCALE * WSCALE
NR = 4           # residual-corrected taps (0..3)

_COMPILED = {}


def _build_program():
    import concourse.bass as bass
    import concourse.tile as tile
    from concourse import bacc, mybir

    f32 = mybir.dt.float32
    bf16 = mybir.dt.bfloat16
    fp8 = mybir.dt.float8e4
    f16 = mybir.dt.float16
    Alu = mybir.AluOpType
    Act = mybir.ActivationFunctionType
    DR = mybir.MatmulPerfMode.DoubleRow

    nc = bacc.Bacc(
        "TRN2",
        target_bir_lowering=False,
        debug=False,
        num_devices=N_CORES,
    )

    x_d = nc.dram_tensor("x", [T, B_LOC, C, L], f32, kind="ExternalInput").ap()
    # [ci, k, co_t, ci_t, co] single e4m3 at scale 2^13
    ws_d = nc.dram_tensor("ws", [128, K, CT, 2, 128], fp8, kind="ExternalInput").ap()
    # residuals for taps 0..NR-1, same layout/scale
    wr_d = nc.dram_tensor("wr", [128, NR, CT, 2, 128], fp8, kind="ExternalInput").ap()
    # [co, field, m(rep), co_t]; fields: b', gamma, beta, 2b', b'^2  (b'=2^13 b)
    chan_d = nc.dram_tensor("chan", [128, 5, 2, CT], f32, kind="ExternalInput").ap()
    # block-diagonal group-sum-broadcast matrix (32-channel blocks of ones)
    onesgg_d = nc.dram_tensor("onesgg", [128, 128], f32, kind="ExternalInput").ap()
    y_d = nc.dram_tensor("y", [T, B_LOC, C, L], f16, kind="ExternalOutput").ap()

    with tile.TileContext(nc) as tc:
        with (
            tc.tile_pool(name="singles", bufs=1) as singles,
            tc.tile_pool(name="xp", bufs=10) as xp,
            tc.tile_pool(name="sp", bufs=6) as sp,
            tc.tile_pool(name="ysb", bufs=8) as ysb,
            tc.tile_pool(name="ysb32", bufs=8) as ysb32,
            tc.tile_pool(name="smallsb", bufs=12) as smallsb,
            tc.tile_pool(name="ypsum", bufs=6, space="PSUM") as ypsum,
            tc.tile_pool(name="spsum", bufs=2, space="PSUM") as spsum,
        ):
            # PE p-state warmup (overlaps startup DMA; keeps clock ramping)
            warm_sb = singles.tile([128, 64], bf16)
            nc.vector.memset(warm_sb[:], 0.25)
            warm_ps = spsum.tile([128, 32], f32, name="small_ps")
            for _ in range(60):
                nc.tensor.matmul(
                    warm_ps[0:64, 0:8], warm_sb[:, 0:64], warm_sb[:, 0:8],
                    start=True, stop=True, skip_group_check=True,
                )
            first_small_ps = warm_ps
            # first x tiles split in halves across queues (gate LIF(0)),
            # then weights (smaller, land in parallel)
            early_x = {}
            for b in range(2):
                xt = xp.tile([128, 2, L], f32)
                xsrc = x_d[0, b].rearrange("(i p) l -> p i l", p=128)
                for h in range(2):
                    for q in range(4):
                        sl = slice(q * (L // 4), (q + 1) * (L // 4))
                        nc.sync.dma_start(
                            out=xt[:, h : h + 1, sl], in_=xsrc[:, h : h + 1, sl]
                        )
                early_x[(0, b)] = xt
            ws = singles.tile([128, K, CT, 2, 128], fp8)
            for k in range(K):
                nc.sync.dma_start(out=ws[:, k], in_=ws_d[:, k])
            wr = singles.tile([128, NR, CT, 2, 128], fp8)
            for k in range(NR):
                nc.sync.dma_start(out=wr[:, k], in_=wr_d[:, k])
            onesgg = singles.tile([128, 128], f32)
            nc.sync.dma_start(out=onesgg[:], in_=onesgg_d[:])
            chan = singles.tile([128, 5, 2, CT], f32)
            nc.sync.dma_start(out=chan[:], in_=chan_d[:])
            eps_t = singles.tile([128, 1], f32)
            nc.vector.memset(eps_t[:], EPS_S)
            # pre-load the activation table (Sqrt selects sqrt_and_others,
            # which also holds Copy/Identity/Square) off the critical path
            eps_s = singles.tile([128, 1], f32)
            nc.scalar.activation(out=eps_s[0:1], in_=eps_t[0:1], func=Act.Sqrt)

            # persistent LIF membrane state (u = 2v scaling) per local batch
            m_tiles = []
            for b in range(B_LOC):
                mt = singles.tile([128, 2, L], f32, tag=f"m{b}")
                m_tiles.append(mt)

            # tap -> (rhs_lo, rhs_hi, out_lo, out_hi) column ranges
            tap_slices = []
            for k in range(K):
                d = k - 2
                if d >= 0:
                    tap_slices.append((d, L, 0, L - d))
                else:
                    tap_slices.append((0, L + d, -d, L))

            mm_list = [("s", 2), ("s", 0), ("s", 1), ("s", 3), ("s", 4)]
            mm_list += [("r", j) for j in range(NR)]
            n_mm = len(mm_list)

            INV_L = 1.0 / L
            INV_G = 1.0 / GPC

            def flush_stats(fs):
                """Stats + PSUM drain for one sample, one iteration later.
                ct0: DVE bn_stats/aggr -> (mean, var); ACT copy -> SBUF.
                ct1: ACT copy+accum (r) -> SBUF; ACT square+accum (q) in PSUM.
                """
                cur, mi, yps, y32s = fs
                stats = cur[2]
                # ct0
                bns = smallsb.tile([128, 6], f32)
                nc.vector.bn_stats(out=bns[:], in_=yps[0][:])
                nc.vector.bn_aggr(out=stats[:, mi, 0, :], in_=bns[:])
                nc.scalar.activation(out=y32s[0][:], in_=yps[0][:], func=Act.Copy)
                # ct1
                nc.scalar.activation(
                    out=y32s[1][:], in_=yps[1][:], func=Act.Copy,
                    accum_out=stats[:, mi, 1, 0:1],
                )
                nc.scalar.activation(
                    out=yps[1][:], in_=yps[1][:], func=Act.Square,
                    accum_out=stats[:, mi, 1, 1:2],
                )

            def tail_front(pend):
                """a/z per-channel stats (DVE) + group-sum-broadcast matmul."""
                tb_pair, small_ps, stats, az, y32all = pend
                gm = stats.shape[1]
                # ct0: a = mean + b' ; z = a^2 + var
                a0 = az[:, :, 0, 0]
                nc.vector.tensor_add(out=a0, in0=stats[:, :, 0, 0], in1=chan[:, 0, 0:gm, 0])
                m0 = smallsb.tile([128, gm], f32)
                nc.vector.tensor_mul(out=m0[:], in0=a0, in1=a0)
                nc.vector.tensor_add(out=az[:, :, 0, 1], in0=m0[:], in1=stats[:, :, 0, 1])
                # ct1: mean = r/L ; a = mean + b' ; z = q/L + b'*(mean + a)
                mc = smallsb.tile([128, gm, 2], f32)
                nc.vector.tensor_scalar(
                    out=mc[:, :, 0], in0=stats[:, :, 1, 0], scalar1=INV_L,
                    scalar2=None, op0=Alu.mult,
                )
                a1 = az[:, :, 1, 0]
                nc.vector.tensor_add(out=a1, in0=mc[:, :, 0], in1=chan[:, 0, 0:gm, 1])
                u1 = smallsb.tile([128, gm, 2], f32)
                nc.vector.tensor_add(out=u1[:, :, 0], in0=mc[:, :, 0], in1=a1)
                nc.vector.tensor_mul(out=u1[:, :, 1], in0=u1[:, :, 0], in1=chan[:, 0, 0:gm, 1])
                nc.vector.scalar_tensor_tensor(
                    out=az[:, :, 1, 1], in0=stats[:, :, 1, 1], scalar=INV_L,
                    in1=u1[:, :, 1], op0=Alu.mult, op1=Alu.add,
                )
                # group sums broadcast to channels: [128, gm*CT*2]
                nc.tensor.matmul(
                    small_ps[:, 0 : gm * 4],
                    onesgg[:],
                    az[:].rearrange("p m c s -> p (m c s)"),
                    start=True, stop=True,
                )

            def tail_mid(pend):
                """mu/kappa chain + A/B coefficients (all 128-partition)."""
                tb_pair, small_ps, stats, az, y32all = pend
                gm = stats.shape[1]
                gsb = small_ps[:, 0 : gm * 4].rearrange("p (m c s) -> p m c s", m=gm, c=CT)
                mu = smallsb.tile([128, gm, CT], f32)
                m2 = smallsb.tile([128, gm, CT], f32)
                vr = smallsb.tile([128, gm, CT], f32)
                kp = smallsb.tile([128, gm, CT], f32)
                nc.vector.tensor_scalar(
                    out=mu[:], in0=gsb[:, :, :, 0], scalar1=INV_G,
                    scalar2=None, op0=Alu.mult,
                )
                nc.vector.tensor_mul(out=m2[:], in0=mu[:], in1=mu[:])
                nc.vector.scalar_tensor_tensor(
                    out=vr[:], in0=gsb[:, :, :, 1], scalar=INV_G, in1=m2[:],
                    op0=Alu.mult, op1=Alu.subtract,
                )
                nc.scalar.activation(
                    out=vr[:], in_=vr[:], func=Act.Sqrt, bias=eps_t[:],
                )
                nc.vector.reciprocal(out=kp[:], in_=vr[:])
                # A = kappa * gamma ; B = (b' - mu) * A + beta  (GpSimd)
                ab = smallsb.tile([128, gm, CT, 2], f32)
                tmp = smallsb.tile([128, gm, CT], f32)
                nc.vector.tensor_mul(out=ab[:, :, :, 0], in0=kp[:], in1=chan[:, 1, 0:gm])
                nc.vector.tensor_sub(out=tmp[:], in0=chan[:, 0, 0:gm], in1=mu[:])
                nc.vector.tensor_mul(out=tmp[:], in0=tmp[:], in1=ab[:, :, :, 0])
                nc.vector.tensor_add(out=ab[:, :, :, 1], in0=tmp[:], in1=chan[:, 2, 0:gm])
                return ab

            def tail_store(pend, ab):
                """out = A*Y + B on ScalarE (f32 SBUF -> fp16 SBUF), DMA out."""
                tb_pair, small_ps, stats, az, y32all = pend
                for mi in range(len(tb_pair)):
                    t, b = tb_pair[mi]
                    last = t == T - 1 and b == B_LOC - 1
                    for ct in range(CT):
                        y_sb = ysb.tile([128, L], f16)
                        nc.scalar.activation(
                            out=y_sb[:], in_=y32all[mi][ct][:], func=Act.Identity,
                            scale=ab[:, mi, ct, 0:1], bias=ab[:, mi, ct, 1:2],
                        )
                        dst = y_d[t, b].rearrange("(i p) l -> p i l", p=128)[:, ct, :]
                        if last:
                            # halves on separate queues to shorten the drain
                            nc.sync.dma_start(out=dst[:, 0 : L // 2], in_=y_sb[:, 0 : L // 2])
                            nc.sync.dma_start(out=dst[:, L // 2 : L], in_=y_sb[:, L // 2 : L])
                        else:
                            nc.sync.dma_start(out=dst, in_=y_sb[:])

            groups = [(2 * i, 2 * i + 1) for i in range(15)] + [(30,), (31,)]
            gof = {}
            for g in groups:
                for j, s_ in enumerate(g):
                    gof[s_] = (g, j)
            pend_stats = None
            pend_reset = None
            pending = None     # group with stats complete, awaiting front
            pend_mid = None    # group with gsum done, awaiting mid+store
            cur = None
            for t in range(T):
                for b in range(B_LOC):
                    idx = t * B_LOC + b
                    grp, mi = gof[idx]
                    gsz = len(grp)
                    if mi == 0:
                        if idx == 0:
                            small_ps = first_small_ps
                        else:
                            small_ps = spsum.tile([128, 32], f32, name="small_ps")
                        stats = smallsb.tile([128, gsz, CT, 2], f32, name="stats")
                        az = smallsb.tile([128, gsz, CT, 2], f32, name="az")
                        cur = ([None] * gsz, small_ps, stats, az, [None] * gsz)
                    cur[0][mi] = (t, b)

                    # 1. LIF for this sample (feeds PE soonest)
                    xt = early_x.pop((t, b), None)
                    if xt is None:
                        xt = xp.tile([128, 2, L], f32)
                        nc.sync.dma_start(
                            out=xt[:],
                            in_=x_d[t, b].rearrange("(i p) l -> p i l", p=128),
                        )
                    mt = m_tiles[b]
                    st = sp.tile([128, 2, L], fp8)
                    if t == 0:
                        nc.vector.tensor_scalar(
                            out=st[:], in0=xt[:], scalar1=1.0, scalar2=None,
                            op0=Alu.is_ge,
                        )
                        nc.vector.scalar_tensor_tensor(
                            out=mt[:], in0=xt[:], scalar=1.0, in1=xt[:],
                            op0=Alu.is_lt, op1=Alu.mult,
                        )
                    else:
                        nc.vector.scalar_tensor_tensor(
                            out=mt[:], in0=mt[:], scalar=0.5, in1=xt[:],
                            op0=Alu.mult, op1=Alu.add,
                        )
                        nc.vector.tensor_scalar(
                            out=st[:], in0=mt[:], scalar1=1.0, scalar2=None,
                            op0=Alu.is_ge,
                        )

                    # 2. stats/drain for previous sample
                    if pend_stats is not None:
                        flush_stats(pend_stats)
                        fcur, fmi, _, fy32 = pend_stats
                        fcur[4][fmi] = fy32
                        if fmi == len(fcur[0]) - 1:
                            pending = fcur
                        pend_stats = None
                    # deferred membrane reset for the previous sample (keeps
                    # this sample's spike at the head of the DVE queue; m[b]
                    # is not needed again for 8 iterations)
                    if pend_reset is not None:
                        nc.vector.scalar_tensor_tensor(
                            out=pend_reset[:], in0=pend_reset[:], scalar=1.0,
                            in1=pend_reset[:], op0=Alu.is_lt, op1=Alu.mult,
                        )
                        pend_reset = None
                    if 0 < t < T - 1:
                        pend_reset = mt

                    # 3. finish the pair before that
                    if pend_mid is not None:
                        ab = tail_mid(pend_mid)
                        tail_store(pend_mid, ab)
                        pend_mid = None

                    # 4. conv for this sample
                    yps = []
                    y32s = []
                    for ct in range(CT):
                        yp = ypsum.tile([128, L], f32)
                        for i, (kind, k) in enumerate(mm_list):
                            rl, rh, ol, oh = tap_slices[k]
                            w_ap = ws[:, k, ct] if kind == "s" else wr[:, k, ct]
                            nc.tensor.matmul(
                                yp[:, ol:oh],
                                w_ap,
                                st[:, :, rl:rh],
                                start=(i == 0),
                                stop=(i == n_mm - 1),
                                perf_mode=DR,
                                skip_group_check=True,
                            )
                        yps.append(yp)
                        y32s.append(ysb32.tile([128, L], f32, name="y32"))
                    pend_stats = (cur, mi, yps, y32s)

                    # 5. group-sum matmul for the completed pair (after this
                    # sample's convs in the PE queue)
                    if mi == 0 and pending is not None:
                        tail_front(pending)
                        pend_mid = pending
                        pending = None

            # final drain
            flush_stats(pend_stats)
            fcur, fmi, _, fy32 = pend_stats
            fcur[4][fmi] = fy32
            if pend_mid is not None:
                ab = tail_mid(pend_mid)
                tail_store(pend_mid, ab)
            tail_front(fcur)
            ab = tail_mid(fcur)
            tail_store(fcur, ab)

    nc.compile()
    return nc


def _prep_host_inputs(x, conv_w, conv_b, gamma, beta):
    x = np.asarray(x, dtype=np.float32)
    conv_w = np.asarray(conv_w, dtype=np.float32)
    conv_b = np.asarray(conv_b, dtype=np.float32)
    gamma = np.asarray(gamma, dtype=np.float32)
    beta = np.asarray(beta, dtype=np.float32)

    def q8(a):
        return a.astype(ml_dtypes.float8_e4m3).astype(np.float32)

    # [ci_t, ci, co_t, co, k] at scale 2^13
    Wt = conv_w.transpose(1, 0, 2)                      # [ci_g, co_g, k]
    W6 = Wt.reshape(2, 128, CT, 128, K) * np.float32(WSCALE)
    w8 = q8(W6)
    r8 = q8(W6 - w8)
    # ws[ci, k, ct, ci_t, co]
    ws_host = np.ascontiguousarray(
        w8.transpose(1, 4, 2, 0, 3).astype(ml_dtypes.float8_e4m3)
    )
    # wr[ci, j(tap), ct, ci_t, co] for taps 0..NR-1
    wr_host = np.ascontiguousarray(
        r8[:, :, :, :, 0:NR].transpose(1, 4, 2, 0, 3).astype(ml_dtypes.float8_e4m3)
    )

    bp = conv_b * np.float32(WSCALE)
    fields = np.stack([bp, gamma, beta, 2.0 * bp, bp * bp])        # [5, 256]
    chan1 = fields.reshape(5, CT, 128).transpose(2, 0, 1)          # [128, 5, ct]
    chan = np.ascontiguousarray(
        np.broadcast_to(chan1[:, :, None, :], (128, 5, 2, CT))
    )

    onesgg = np.zeros((128, 128), np.float32)
    for ci in range(128):
        g0 = (ci // GPC) * GPC
        onesgg[ci, g0 : g0 + GPC] = 1.0

    shards = []
    for i in range(N_CORES):
        shards.append(
            {
                "x": np.ascontiguousarray(x[:, i * B_LOC : (i + 1) * B_LOC]),
                "ws": ws_host,
                "wr": wr_host,
                "chan": chan,
                "onesgg": onesgg,
            }
        )
    return shards


def kernel(x, conv_w, conv_b, gamma, beta, _trace=False):
    from concourse.bass_utils import run_bass_kernel_spmd

    if "nc" not in _COMPILED:
        _COMPILED["nc"] = _build_program()
    nc = _COMPILED["nc"]

    in_maps = _prep_host_inputs(x, conv_w, conv_b, gamma, beta)
    res = run_bass_kernel_spmd(
        nc, in_maps, list(range(N_CORES)), trace=_trace
    )
    out = np.concatenate([r["y"] for r in res.results], axis=1).astype(np.float32)
    _COMPILED["last_result"] = res
    return out
